# revision 36
# baseline (speedup 1.0000x reference)
"""Trainium2 Bass kernel for nn_Conv1dMapper (3x conv1d+bn -> 3x fc+bn -> interp epilogue).

Self-contained: accepts FULL inputs, shards across 8 NeuronCores internally,
returns the FULL [64, 12, 100] output.

Sharding strategy:
  - conv stage (tiny) replicated on all cores; bn1/bn2 folded into the next
    conv's weights at runtime; conv3 packs even/odd output positions into
    128 partitions via PE column-tiling so fc1 gets K=128 contraction chunks.
  - fc1 row-sharded (1250 -> padded 1280 rows/core), bn4 local, AllGather.
  - fc2 row-sharded (full 10240-padded contraction), bn5 local.
  - fc3 contraction-sharded with the output epilogue (channel zero/one + linear
    interpolation, which is linear in fc3's output) folded into fc3's weights
    on the host; partials summed with ReduceScatter.

Perf notes:
  - all large weights and matmul activations are bf16 (PSUM accumulation and
    bn statistics stay fp32): halves the HBM weight traffic and runs the PE
    at 4x the fp32 rate.
  - DMA instruction count is minimized (the per-DMA HWDGE setup cost ~625ns
    serializes on one engine): weights stream in 2-chunk [128, 2560] tiles,
    small constants ride in packed tensors, fc3's weights load in one DMA.
  - fc1/fc2 run single-pass with 10 concurrent 64-col accumulation groups
    packed into two PSUM banks ([128,512] + [128,128]).
"""

import sys

sys.path.insert(0, "/opt/trn_rl_repo")

import numpy as np
import ml_dtypes

BF16 = np.dtype(ml_dtypes.bfloat16)

N_CORES = 8
B = 64            # batch
L1, L2, L3 = 98, 96, 94
NCH = 64          # conv channels
H = 6016          # fc1 in features = 64*94
L3H = 47          # = L3 // 2
HID = 10000
PREAL = 1250      # fc1/fc2 rows per core
PCORE = 1280      # padded rows per core
HIDP = PCORE * N_CORES  # 10240
KK2 = HIDP // 128       # 80 fc2 contraction chunks
OUTF = 1200
OCORE = OUTF // N_CORES  # 150
EPS = 1e-5

_CACHE = {}


# ---------------------------------------------------------------- host prep

def _fold_epilogue(fc3_w, fc3_b):
    """Fold reshape->zero/one channels->interpolation into fc3's weights."""
    L = 100
    CD = L // 3
    CPS = np.array([1, CD, 2 * CD, 3 * CD])
    REG = np.array([p for p in range(L) if p not in set(CPS.tolist())][1:])
    J = REG // CD
    Lp = CPS[J]
    Rp = CPS[J + 1]
    ALPHA = ((REG - Lp) / CD).astype(np.float32)
    CH = np.array([0, 2, 8, 10, 3, 11])
    ZERO_CH = np.array([1, 4, 6, 7, 9])

    W3e = fc3_w.astype(np.float32).copy()
    b3e = fc3_b.astype(np.float32).copy()
    idx0 = (ZERO_CH[:, None] * L + np.arange(L)[None, :]).ravel()
    W3e[idx0] = 0.0
    b3e[idx0] = 0.0
    idx1 = 5 * L + np.arange(L)
    W3e[idx1] = 0.0
    b3e[idx1] = 1.0
    rows_t = (CH[:, None] * L + REG[None, :]).ravel()
    rows_l = (CH[:, None] * L + Lp[None, :]).ravel()
    rows_r = (CH[:, None] * L + Rp[None, :]).ravel()
    a = np.broadcast_to(ALPHA[None, :], (len(CH), len(REG))).ravel()[:, None]
    W3e[rows_t] = a * fc3_w[rows_l] + (1.0 - a) * fc3_w[rows_r]
    b3e[rows_t] = (a[:, 0] * fc3_b[rows_l] + (1.0 - a[:, 0]) * fc3_b[rows_r])
    return W3e, b3e


def _prep_in_maps(inp):
    f32 = np.float32
    x = np.asarray(inp["x"], f32)

    # conv1 im2col: X9[k*3+i, l*64+b] = x[b, i, l+k]; conv1 weights ride in
    # the last 64 columns of the same tensor (one DMA).
    x_t = np.ascontiguousarray(x.transpose(1, 2, 0))      # [3, 100, 64]
    X9 = np.stack([x_t[:, k:k + L1, :] for k in range(3)], 0)  # [k, i, l, b]
    X9 = X9.reshape(9, L1 * B)
    X9 = np.concatenate([X9, np.zeros((23, L1 * B), f32)], 0)
    w1 = np.asarray(inp["conv1_w"], f32).transpose(2, 1, 0).reshape(9, NCH)
    w1 = np.concatenate([w1, np.zeros((23, NCH), f32)], 0)
    X9w1 = np.ascontiguousarray(
        np.concatenate([X9, w1], 1)).astype(BF16)         # [32, 6336]

    w2 = np.asarray(inp["conv2_w"], f32).transpose(1, 2, 0).reshape(NCH, 3 * NCH)
    w3 = np.asarray(inp["conv3_w"], f32).transpose(1, 2, 0).reshape(NCH, 3 * NCH)
    w23 = np.ascontiguousarray(np.concatenate([w2, w3], 1))  # [64, 384]

    # packed per-channel vectors: cb1 cb2 cb3 g1 be1 g2 be2 g3 be3
    cvec = np.ascontiguousarray(np.stack(
        [np.asarray(inp[k], f32) for k in
         ("conv1_b", "conv2_b", "conv3_b", "bn1_g", "bn1_b",
          "bn2_g", "bn2_b", "bn3_g", "bn3_b")], 1))       # [64, 9]

    common = {"X9w1": X9w1, "w23": w23, "cvec": cvec}

    fc1_w = np.asarray(inp["fc1_w"], f32)
    fc2_w = np.asarray(inp["fc2_w"], f32)
    W3e, b3e = _fold_epilogue(np.asarray(inp["fc3_w"], f32),
                              np.asarray(inp["fc3_b"], f32))

    def padrows(a, n):
        return np.concatenate([a, np.zeros((n - a.shape[0],) + a.shape[1:], f32)], 0)

    def tiles128(vec):  # [1280] -> [128, 10]
        return vec.reshape(10, 128).T

    # fc2 column padding map: padded q = 1280*r + jj <-> real 1250*r + jj
    valid_q = (PCORE * np.arange(N_CORES)[:, None] + np.arange(PREAL)[None, :]).ravel()
    src_f = (PREAL * np.arange(N_CORES)[:, None] + np.arange(PREAL)[None, :]).ravel()

    # fc2 chunk processing order: the 40 chunks touching fc1's first-half
    # features (m-tile < 5 within each source core's block) first, so fc2 can
    # run on the first AllGather's payload while the second is in flight.
    ORDER = ([10 * g + m for g in range(N_CORES) for m in range(5)]
             + [10 * g + 5 + m for g in range(N_CORES) for m in range(5)])

    in_maps = []
    for c in range(N_CORES):
        m = dict(common)
        r0 = PREAL * c
        # fc1 shard [47, 128, 1280] -> two half-column streams of 12 tiles
        # [128, 2560] (4 contraction chunks x 640 cols, host-merged so each
        # partition's DMA descriptor is one contiguous 5KB run).
        blk = padrows(fc1_w[r0:r0 + PREAL], PCORE)        # [1280, 6016]
        v = blk.reshape(PCORE, NCH, L3H, 2).transpose(2, 3, 1, 0)
        W1P = np.concatenate([v.reshape(L3H, 128, PCORE),
                              np.zeros((1, 128, PCORE), f32)], 0)  # [48,128,1280]
        for half, key in ((0, "W1A"), (1, "W1B")):
            hw = W1P[:, :, 640 * half:640 * half + 640]   # [48, 128, 640]
            m[key] = np.ascontiguousarray(
                hw.reshape(12, 4, 128, 640).transpose(0, 2, 1, 3)
                .reshape(12, 128, 2560)).astype(BF16)
        # packed [128, 30]: fb1 | g4 | be4
        m["fgb4"] = np.ascontiguousarray(np.concatenate(
            [tiles128(padrows(np.asarray(inp[k], f32)[r0:r0 + PREAL], PCORE))
             for k in ("fc1_b", "bn4_g", "bn4_b")], 1))

        # fc2 shard: chunks in ORDER, host-merged in pairs: [40, 128, 2560]
        A = padrows(fc2_w[r0:r0 + PREAL], PCORE)          # [1280, 10000]
        Bm = np.zeros((PCORE, HIDP), f32)
        Bm[:, valid_q] = A[:, src_f]
        W2P = Bm.reshape(PCORE, KK2, 128).transpose(1, 2, 0)  # [80, 128, 1280]
        m["W2Q"] = np.ascontiguousarray(
            W2P[ORDER].reshape(40, 2, 128, PCORE).transpose(0, 2, 1, 3)
            .reshape(40, 128, 2 * PCORE)).astype(BF16)
        fgb5 = np.concatenate(
            [tiles128(padrows(np.asarray(inp[k], f32)[r0:r0 + PREAL], PCORE))
             for k in ("fc2_b", "bn5_g", "bn5_b")], 1)
        # padded feature 1250 (partition 98, m-tile 9) acts as fc3's bias
        # slot: its bn5 scale is 0 (padded gamma), so be5=1 makes the bn5
        # apply write an exact 1.0 there.
        fgb5[98, 29] = 1.0
        m["fgb5"] = np.ascontiguousarray(fgb5)

        # fc3 contraction shard, host-merged to [128, 12000]. The bias rides
        # contraction slot 1250 (a padding row): h2n's matching slot is 1.0.
        cols = W3e[:, r0:r0 + PREAL]                      # [1200, 1250]
        colsp = np.concatenate([cols, np.zeros((OUTF, PCORE - PREAL), f32)], 1)
        colsp[:, PREAL] = b3e / N_CORES
        m["W3Q"] = np.ascontiguousarray(
            colsp.T.reshape(10, 128, OUTF).transpose(1, 0, 2)
            .reshape(128, 10 * OUTF)).astype(BF16)
        in_maps.append(m)
    return in_maps


# ---------------------------------------------------------------- device build

def _build_nc(single_core_prof=False):
    """single_core_prof: build a 1-core variant with collectives replaced by
    cost-equivalent local DMAs, for TimelineSim estimates only."""
    from concourse import bacc, tile, mybir

    dt = mybir.dt.float32
    bt = mybir.dt.bfloat16
    AF = mybir.ActivationFunctionType
    AL = mybir.AluOpType

    nc = bacc.Bacc("TRN2", target_bir_lowering=False, debug=False,
                   num_devices=1 if single_core_prof else N_CORES)

    def din(name, shape, dd=dt):
        return nc.dram_tensor(name, list(shape), dd, kind="ExternalInput").ap()

    X9w1 = din("X9w1", [32, L1 * B + NCH], bt)
    w23 = din("w23", [NCH, 6 * NCH])
    cvec = din("cvec", [NCH, 9])
    W1A = din("W1A", [12, 128, 2560], bt)
    W1B = din("W1B", [12, 128, 2560], bt)
    fgb4 = din("fgb4", [128, 30])
    W2Q = din("W2Q", [40, 128, 2 * PCORE], bt)
    fgb5 = din("fgb5", [128, 30])
    W3Q = din("W3Q", [128, 10 * OUTF], bt)
    out = nc.dram_tensor("out", [OCORE, B], dt, kind="ExternalOutput").ap()

    with tile.TileContext(nc) as tc:
        with (tc.tile_pool(name="const", bufs=1) as cst,
              tc.tile_pool(name="acts", bufs=1) as acts,
              tc.tile_pool(name="wst", bufs=16) as wst,
              tc.tile_pool(name="scr", bufs=2) as scrp,
              tc.tile_pool(name="dram", bufs=1, space="DRAM") as dram):

            # ---- load constants / small tensors (batched DMAs)
            X9s = cst.tile([32, L1 * B + NCH], bt)
            nc.sync.dma_start(X9s[:], X9w1[:])
            w1s = X9s[:, L1 * B:L1 * B + NCH]
            w23s = cst.tile([NCH, 6 * NCH], dt)
            nc.sync.dma_start(w23s[:], w23[:])
            w2s = w23s[:, 0:3 * NCH]
            w3s = w23s[:, 3 * NCH:6 * NCH]
            cvs = cst.tile([NCH, 9], dt)
            nc.sync.dma_start(cvs[:], cvec[:])
            cb1s, cb2s, cb3s = cvs[:, 0:1], cvs[:, 1:2], cvs[:, 2:3]
            g1s, be1s = cvs[:, 3:4], cvs[:, 4:5]
            g2s, be2s = cvs[:, 5:6], cvs[:, 6:7]
            g3s, be3s = cvs[:, 7:8], cvs[:, 8:9]
            fgb4s = cst.tile([128, 30], dt)
            nc.sync.dma_start(fgb4s[:], fgb4[:])
            fb1s, g4s, be4s = fgb4s[:, 0:10], fgb4s[:, 10:20], fgb4s[:, 20:30]
            fgb5s = cst.tile([128, 30], dt)
            nc.sync.dma_start(fgb5s[:], fgb5[:])
            fb2s, g5s, be5s = fgb5s[:, 0:10], fgb5s[:, 10:20], fgb5s[:, 20:30]
            epsb = cst.tile([128, 1], dt, name="epsb")
            nc.vector.memset(epsb[:], EPS)

            # ---- activations
            U1 = acts.tile([NCH, L1 * B], bt)       # relu(conv1+b)
            U2 = acts.tile([NCH, L2 * B], bt)       # relu(conv2'+b2')
            U3 = acts.tile([128, L3H * B], bt)      # bn3(relu(conv3'+b3')), parity-packed
            # fc1 runs in two column halves (m-tiles 0..4 / 5..9) so the first
            # AllGather overlaps the second half's compute; tiles are split
            # per half so dependency tracking stays precise.
            h1A = acts.tile([128, 5 * B], dt)       # fc1 half A pre-bn
            h1B = acts.tile([128, 5 * B], dt)
            h1nA = acts.tile([128, 5 * B], bt)      # bn4 out, half A
            h1nB = acts.tile([128, 5 * B], bt)
            h1gA = acts.tile([128, 40 * B], bt)     # gathered half A (8 cores x 5 m)
            h1gB = acts.tile([128, 40 * B], bt)
            h2n = acts.tile([128, 10 * B], bt)      # bn5 out

            # stats tiles
            s1sum = cst.tile([NCH, 16], dt); s1sq = cst.tile([NCH, 16], dt)
            s2sum = cst.tile([NCH, 16], dt); s2sq = cst.tile([NCH, 16], dt)
            s3sum = cst.tile([128, 8], dt); s3sq = cst.tile([128, 8], dt)

            def bn_vec(pref, p, n):
                return {k: cst.tile([p, n], dt, tag=f"{pref}_{k}",
                                    name=f"{pref}_{k}")
                        for k in ("S", "Q", "m", "var", "std", "rstd",
                                  "s", "t", "tmp")}

            def bn_scale_shift(d, S_ap, Q_ap, count, g_ap, b_ap):
                """var/rstd/scale/shift from S (sum) and Q (sum of squares)."""
                inv = 1.0 / count
                nc.vector.tensor_scalar_mul(d["m"][:], S_ap, inv)
                nc.vector.tensor_tensor(d["tmp"][:], d["m"][:], d["m"][:], op=AL.mult)
                # var = Q*inv - m^2
                nc.vector.scalar_tensor_tensor(
                    d["var"][:], Q_ap, inv, d["tmp"][:],
                    op0=AL.mult, op1=AL.subtract)
                # rstd = 1/sqrt(var + eps)
                nc.scalar.activation(d["std"][:], d["var"][:], AF.Sqrt,
                                     bias=epsb[0:d["var"].shape[0], :])
                nc.vector.reciprocal(d["rstd"][:], d["std"][:])
                nc.vector.tensor_tensor(d["s"][:], g_ap, d["rstd"][:], op=AL.mult)
                nc.vector.tensor_tensor(d["tmp"][:], d["m"][:], d["s"][:], op=AL.mult)
                nc.vector.tensor_tensor(d["t"][:], b_ap, d["tmp"][:], op=AL.subtract)

            def bn_from_sums(d, sums, sqs, ntile, count, g_ap, b_ap):
                """Per-partition bn scale/shift from per-tile sums."""
                nc.vector.reduce_sum(d["S"][:], sums[:, 0:ntile], axis=mybir.AxisListType.X)
                nc.vector.reduce_sum(d["Q"][:], sqs[:, 0:ntile], axis=mybir.AxisListType.X)
                bn_scale_shift(d, d["S"][:], d["Q"][:], count, g_ap, b_ap)

            # =========================================================
            # conv1: U1 = relu(w1.T @ X9 + cb1)
            ps_cm = tc.tile_pool(name="pscv", bufs=2, space="PSUM")
            ps = ps_cm.__enter__()
            n1 = L1 * B  # 6272
            t1sizes = [512] * 12 + [128]
            for t in range(13):
                sz = t1sizes[t]
                pt = ps.tile([NCH, 512], dt, tag="cps")
                nc.tensor.matmul(pt[:, 0:sz], w1s, X9s[:, 512 * t:512 * t + sz],
                                 start=True, stop=True)
                nc.scalar.activation(U1[:, 512 * t:512 * t + sz], pt[:, 0:sz],
                                     AF.Relu, bias=cb1s, accum_out=s1sum[:, t:t + 1])
                sc = scrp.tile([128, 512], dt, tag="scr", name="sc")
                nc.vector.tensor_tensor_reduce(
                    sc[0:NCH, 0:sz], U1[:, 512 * t:512 * t + sz], U1[:, 512 * t:512 * t + sz],
                    scale=1.0, scalar=0.0, op0=AL.mult, op1=AL.add,
                    accum_out=s1sq[:, t:t + 1])

            bn1 = bn_vec("bn1", NCH, 1)
            bn_from_sums(bn1, s1sum, s1sq, 13, float(n1), g1s, be1s)

            # fold bn1 into conv2 weights: w2f = w2 * s1 (per in-channel),
            # b2f = cb2 + sum_k w2[k].T @ t1
            w2f = cst.tile([NCH, 3 * NCH], bt)
            nc.vector.tensor_scalar_mul(w2f[:], w2s, bn1["s"][:])
            pb = ps.tile([NCH, 1], dt, tag="cpb")
            for k in range(3):
                nc.tensor.matmul(pb[:], w2s[:, 64 * k:64 * k + 64], bn1["t"][:],
                                 start=(k == 0), stop=(k == 2))
            b2f = cst.tile([NCH, 1], dt)
            nc.vector.tensor_tensor(b2f[:], pb[:], cb2s, op=AL.add)

            # =========================================================
            # conv2: U2 = relu(w2f.T conv U1 + b2f)
            for t in range(12):
                pt = ps.tile([NCH, 512], dt, tag="cps")
                for k in range(3):
                    nc.tensor.matmul(pt[:], w2f[:, 64 * k:64 * k + 64],
                                     U1[:, (8 * t + k) * B:(8 * t + k) * B + 512],
                                     start=(k == 0), stop=(k == 2))
                nc.scalar.activation(U2[:, 512 * t:512 * t + 512], pt[:],
                                     AF.Relu, bias=b2f[:], accum_out=s2sum[:, t:t + 1])
                sc = scrp.tile([128, 512], dt, tag="scr", name="sc")
                nc.vector.tensor_tensor_reduce(
                    sc[0:NCH, :], U2[:, 512 * t:512 * t + 512], U2[:, 512 * t:512 * t + 512],
                    scale=1.0, scalar=0.0, op0=AL.mult, op1=AL.add,
                    accum_out=s2sq[:, t:t + 1])

            bn2 = bn_vec("bn2", NCH, 1)
            bn_from_sums(bn2, s2sum, s2sq, 12, float(L2 * B), g2s, be2s)

            w3f = cst.tile([NCH, 3 * NCH], bt)
            nc.vector.tensor_scalar_mul(w3f[:], w3s, bn2["s"][:])
            pb3 = ps.tile([NCH, 1], dt, tag="cpb")
            for k in range(3):
                nc.tensor.matmul(pb3[:], w3s[:, 64 * k:64 * k + 64], bn2["t"][:],
                                 start=(k == 0), stop=(k == 2))
            b3f = cst.tile([NCH, 1], dt)
            nc.vector.tensor_tensor(b3f[:], pb3[:], cb3s, op=AL.add)
            b3d = cst.tile([128, 1], dt)
            nc.vector.tensor_copy(b3d[0:NCH, :], b3f[:])
            nc.vector.tensor_copy(b3d[NCH:128, :], b3f[:])

            # =========================================================
            # conv3 (parity-packed): U3[par*64+c, l2*64+b] = relu(conv3')
            U2v = U2[:].rearrange("p (l two b) -> p two l b", two=2, b=B)
            t3l2 = [8, 8, 8, 8, 8, 7]   # 47 l2 positions
            for t in range(6):
                lw = t3l2[t]
                pt = ps.tile([128, 512], dt, tag="cps3")
                for par in range(2):
                    for k in range(3):
                        pk = par + k
                        rhs = U2v[:, pk % 2, 8 * t + pk // 2: 8 * t + pk // 2 + lw, :]
                        nc.tensor.matmul(pt[64 * par:64 * par + 64, 0:64 * lw],
                                         w3f[:, 64 * k:64 * k + 64], rhs,
                                         start=(k == 0), stop=(k == 2),
                                         tile_position=(0, 64 * par))
                nc.scalar.activation(U3[:, 512 * t:512 * t + 64 * lw], pt[:, 0:64 * lw],
                                     AF.Relu, bias=b3d[:], accum_out=s3sum[:, t:t + 1])
                sc = scrp.tile([128, 512], dt, tag="scr")
                nc.vector.tensor_tensor_reduce(
                    sc[:, 0:64 * lw], U3[:, 512 * t:512 * t + 64 * lw],
                    U3[:, 512 * t:512 * t + 64 * lw],
                    scale=1.0, scalar=0.0, op0=AL.mult, op1=AL.add,
                    accum_out=s3sq[:, t:t + 1])

            # bn3: combine parity halves, then broadcast back to 128 partitions
            S3 = cst.tile([128, 1], dt); Q3 = cst.tile([128, 1], dt)
            nc.vector.reduce_sum(S3[:], s3sum[:, 0:6], axis=mybir.AxisListType.X)
            nc.vector.reduce_sum(Q3[:], s3sq[:, 0:6], axis=mybir.AxisListType.X)
            cS = cst.tile([NCH, 1], dt); cQ = cst.tile([NCH, 1], dt)
            nc.vector.tensor_copy(cS[:], S3[NCH:128, :])
            nc.vector.tensor_copy(cQ[:], Q3[NCH:128, :])
            St = cst.tile([NCH, 1], dt); Qt = cst.tile([NCH, 1], dt)
            nc.vector.tensor_tensor(St[:], S3[0:NCH, :], cS[:], op=AL.add)
            nc.vector.tensor_tensor(Qt[:], Q3[0:NCH, :], cQ[:], op=AL.add)

            bn3 = bn_vec("bn3", NCH, 1)
            bn_scale_shift(bn3, St[:], Qt[:], float(H), g3s, be3s)
            s3b = cst.tile([128, 1], dt); t3b = cst.tile([128, 1], dt)
            nc.vector.tensor_copy(s3b[0:NCH, :], bn3["s"][:])
            nc.vector.tensor_copy(s3b[NCH:128, :], bn3["s"][:])
            nc.vector.tensor_copy(t3b[0:NCH, :], bn3["t"][:])
            nc.vector.tensor_copy(t3b[NCH:128, :], bn3["t"][:])
            nc.vector.tensor_scalar(U3[:], U3[:], s3b[:], t3b[:],
                                    op0=AL.mult, op1=AL.add)
            ps_cm.__exit__(None, None, None)
            psfc_cm = tc.tile_pool(name="psfc", bufs=2, space="PSUM")
            psfc = psfc_cm.__enter__()

            # =========================================================
            # fc1: h1 = bn4(relu(W1 @ h + b1)), row-sharded, two column
            # halves (m 0..4 then 5..9). Each half: single accumulation pass
            # (5 concurrent 64-col groups in one PSUM bank), bn4 on its own
            # stats, then its own AllGather — so gather A overlaps fc1 half B
            # and gather B overlaps fc2's first 40 chunks.
            halves = []
            for half, (W1H, h1h, h1nh) in enumerate(
                    ((W1A, h1A, h1nA), (W1B, h1B, h1nB))):
                accP = psfc.tile([128, 320], dt, tag="accP", name=f"fc1acc{half}")
                hsum = cst.tile([128, 5], dt, name=f"h1sum{half}")
                hsq = cst.tile([128, 5], dt, name=f"h1sq{half}")
                for t in range(12):
                    W1t = wst.tile([128, 2560], bt, tag="wst", name="W1t")
                    nc.sync.dma_start(W1t[:], W1H[t][:])
                    for q in range(4):
                        cc = 4 * t + q
                        if cc >= L3H:
                            break
                        rhs = U3[:, B * cc:B * cc + B]
                        for m in range(5):
                            # start=True zeroes the whole 2KB PSUM zero-region,
                            # so only the first group per bank starts; siblings
                            # ride the pending-zero semantics.
                            nc.tensor.matmul(
                                accP[:, 64 * m:64 * m + 64],
                                W1t[:, 640 * q + 128 * m:640 * q + 128 * m + 128],
                                rhs, start=(cc == 0 and m == 0),
                                stop=(cc == L3H - 1 and m == 0),
                                skip_group_check=(m != 0))
                for m in range(5):
                    mm = 5 * half + m
                    nc.scalar.activation(h1h[:, 64 * m:64 * m + 64],
                                         accP[:, 64 * m:64 * m + 64],
                                         AF.Relu, bias=fb1s[:, mm:mm + 1],
                                         accum_out=hsum[:, m:m + 1])
                    sc = scrp.tile([128, 512], dt, tag="scr", name="sc")
                    nc.vector.tensor_tensor_reduce(
                        sc[:, 0:64], h1h[:, 64 * m:64 * m + 64],
                        h1h[:, 64 * m:64 * m + 64],
                        scale=1.0, scalar=0.0, op0=AL.mult, op1=AL.add,
                        accum_out=hsq[:, m:m + 1])
                bn4h = bn_vec(f"bn4{half}", 128, 5)
                bn_scale_shift(bn4h, hsum[:], hsq[:], float(B),
                               g4s[:, 5 * half:5 * half + 5],
                               be4s[:, 5 * half:5 * half + 5])
                for m in range(5):
                    nc.vector.tensor_scalar(h1nh[:, 64 * m:64 * m + 64],
                                            h1h[:, 64 * m:64 * m + 64],
                                            bn4h["s"][:, m:m + 1],
                                            bn4h["t"][:, m:m + 1],
                                            op0=AL.mult, op1=AL.add)

                # AllGather this half: core g's rows land at 128g+p; unpack
                # into the compact gathered layout [128, (g m b)].
                # The gather pipeline lives on the Pool queue: a dependent DMA
                # waiting at the head of SP.SEQ would stall the whole weight
                # stream issued behind it.
                agin = dram.tile([128, 320], bt, name=f"agin{half}")
                agout = dram.tile(
                    [128 * N_CORES, 320], bt, name=f"agout{half}",
                    addr_space="Local" if single_core_prof else "Shared")
                nc.gpsimd.dma_start(agin[:], h1nh[:])
                if single_core_prof:
                    for g in range(N_CORES):
                        nc.gpsimd.dma_start(agout[128 * g:128 * (g + 1), :], agin[:])
                else:
                    nc.gpsimd.collective_compute(
                        "AllGather", mybir.AluOpType.bypass,
                        replica_groups=[list(range(N_CORES))],
                        ins=[agin[:]], outs=[agout[:]])
                h1gh = h1gA if half == 0 else h1gB
                nc.gpsimd.dma_start(
                    h1gh[:].rearrange("p (g n) -> p g n", g=8),
                    agout[:].rearrange("(g p) n -> p g n", p=128))
                halves.append(h1gh)

                if half == 0:
                    # prefetch all fc3 weights in one DMA between the two
                    # weight streams so fc3 never waits on DMA at the tail.
                    W3all = cst.tile([128, 10 * OUTF], bt, name="W3all")
                    nc.sync.dma_start(W3all[:], W3Q[:])

            # =========================================================
            # fc2, single pass over the 80 chunks in gather-arrival order
            # (all half-A chunks first), 2 chunks per weight tile.
            h2sum = cst.tile([128, 10], dt); h2sq = cst.tile([128, 10], dt)
            h2 = acts.tile([128, 10 * B], dt)
            accA = psfc.tile([128, 512], dt, tag="accA", name="fc2accA")
            accB = psfc.tile([128, 128], dt, tag="accB", name="fc2accB")

            def accsl(m):
                return (accA[:, 64 * m:64 * m + 64] if m < 8
                        else accB[:, 64 * (m - 8):64 * (m - 8) + 64])

            for t in range(KK2 // 2):
                W2t = wst.tile([128, 2560], bt, tag="wst", name="W2t")
                nc.sync.dma_start(W2t[:], W2Q[t][:])
                for j in range(2):
                    i = 2 * t + j
                    src = halves[i // 40]
                    rhs = src[:, B * (i % 40):B * (i % 40) + B]
                    for m in range(10):
                        nc.tensor.matmul(
                            accsl(m), W2t[:, PCORE * j + 128 * m:PCORE * j + 128 * m + 128],
                            rhs, start=(i == 0 and m in (0, 8)),
                            stop=(i == KK2 - 1 and m in (0, 8)),
                            skip_group_check=(m not in (0, 8)))
            for m in range(10):
                nc.scalar.activation(h2[:, 64 * m:64 * m + 64], accsl(m),
                                     AF.Relu, bias=fb2s[:, m:m + 1],
                                     accum_out=h2sum[:, m:m + 1])
                sc = scrp.tile([128, 512], dt, tag="scr", name="sc")
                nc.vector.tensor_tensor_reduce(
                    sc[:, 0:64], h2[:, 64 * m:64 * m + 64],
                    h2[:, 64 * m:64 * m + 64],
                    scale=1.0, scalar=0.0, op0=AL.mult, op1=AL.add,
                    accum_out=h2sq[:, m:m + 1])

            bn5 = bn_vec("bn5", 128, 10)
            bn_scale_shift(bn5, h2sum[:], h2sq[:], float(B), g5s, be5s)
            for m in range(10):
                nc.vector.tensor_scalar(h2n[:, 64 * m:64 * m + 64],
                                        h2[:, 64 * m:64 * m + 64],
                                        bn5["s"][:, m:m + 1], bn5["t"][:, m:m + 1],
                                        op0=AL.mult, op1=AL.add)
            # (fc3's bias slot — padded feature 1250, partition 98 of m-tile 9
            #  — was set to exactly 1.0 by the bn5 apply via be5; see host prep)

            # =========================================================
            # fc3 partials (bias included via the ones slot), ReduceScatter
            accA = psfc.tile([128, 512], dt, tag="accA", name="fc3accA")
            accB = psfc.tile([128, 128], dt, tag="accB", name="fc3accB")

            def accsl3(m):
                return (accA[0:120, 64 * m:64 * m + 64] if m < 8
                        else accB[0:120, 64 * (m - 8):64 * (m - 8) + 64])

            for kk in range(10):
                rhs = h2n[:, B * kk:B * kk + B]
                for m in range(10):
                    nc.tensor.matmul(
                        accsl3(m), W3all[:, OUTF * kk + 120 * m:OUTF * kk + 120 * m + 120],
                        rhs, start=(kk == 0 and m in (0, 8)),
                        stop=(kk == 9 and m in (0, 8)),
                        skip_group_check=(m not in (0, 8)))

            q3 = acts.tile([120, 10 * B], dt)
            nc.vector.tensor_copy(q3[:, 0:512], accA[0:120, :])
            nc.vector.tensor_copy(q3[:, 512:640], accB[0:120, :])
            rsin = dram.tile([OUTF, B], dt)
            rsout = dram.tile([OCORE, B], dt)
            nc.gpsimd.dma_start(
                rsin[:].rearrange("(m p) b -> p m b", p=120),
                q3[:].rearrange("p (m b) -> p m b", b=B))
            if single_core_prof:
                nc.gpsimd.dma_start(rsout[:], rsin[0:OCORE, :])
            else:
                nc.gpsimd.collective_compute(
                    "ReduceScatter", mybir.AluOpType.add,
                    replica_groups=[list(range(N_CORES))],
                    ins=[rsin[:]], outs=[rsout[:]])
            nc.gpsimd.dma_start(out[:], rsout[:])
            psfc_cm.__exit__(None, None, None)

    nc.compile()
    return nc


# ---------------------------------------------------------------- entry point

def _run_sim(nc, in_maps):
    from concourse.bass_interp import MultiCoreSim

    sim = MultiCoreSim(nc, num_cores=N_CORES, trace=False,
                       require_finite=False, require_nnan=False)
    for i, (cid, core) in enumerate(sim.cores.items()):
        for name, arr in in_maps[i].items():
            core.tensor(name)[:] = arr
    sim.simulate(check_with_hw=False)
    return [np.array(sim.cores[c].tensor("out")) for c in range(N_CORES)]


def kernel(**inputs):
    from concourse import bass_utils

    if "nc" not in _CACHE:
        _CACHE["nc"] = _build_nc()
    nc = _CACHE["nc"]

    in_maps = _prep_in_maps(inputs)
    outs = None
    for attempt in range(2):
        try:
            res = bass_utils.run_bass_kernel_spmd(
                nc, in_maps, core_ids=list(range(N_CORES)))
            outs = [res.results[i]["out"] for i in range(N_CORES)]
            break
        except Exception as e:
            # device may be wedged from a prior run; one retry usually
            # recovers it. After that, fall back to the simulator.
            print(f"kernel: HW attempt {attempt} failed: "
                  f"{type(e).__name__}: {str(e)[:300]}", file=sys.stderr)
            continue
    if outs is None:
        outs = _run_sim(nc, in_maps)
    outT = np.concatenate(outs, 0)
    return np.ascontiguousarray(outT.T).reshape(B, 12, 100).astype(np.float32)
